# revision 1
# baseline (speedup 1.0000x reference)
"""MultiHeadTimeDimensionAttention kernel for Trainium2 (8 NeuronCores).

Math (per batch b):
  q[h,d]      = o_last[b] . Wq[h,:,d] + bq[h,d]
  scores[t,h] = sum_z o_all[b,t,z] * wkq[z,h]        (wkq[z,h] = sum_d Wk[h,z,d] q[h,d])
                (bk folds to a per-head constant -> softmax invariant -> dropped)
  p = exp(scores - max_t), l = sum_t p               (softmax unnormalized)
  r[h,z]      = sum_t p[t,h] * o_all[b,t,z]
  ctx[h,d]    = (sum_z r[h,z] Wv[h,z,d]) / l[h] + bv[h,d]

Exact algebraic restructure of the reference (einsum reassociation), ~64x
fewer FLOPs than materializing K/V. fp16 PE inputs (fp32 PSUM accumulation),
softmax in fp32.

Sharding: data-parallel over B; each of the 8 cores handles B/8=2 batches.
A^T tiles come half from XBAR DMA-transpose (DMA is otherwise idle), half
from PE transposes (PE-side staging in PSUM).
"""

import numpy as np

import concourse.bacc as bacc
import concourse.tile as tile
import concourse.mybir as mybir
from concourse.bass_utils import run_bass_kernel_spmd
from concourse.masks import make_identity

B, T, Z, H, DK = 16, 4096, 1024, 16, 64
P = 128
NCORES = 8
BLOC = B // NCORES          # batches per core
ZC = Z // P                 # 8 z-chunks
NT = T // P                 # 32 t-tiles
TB = 512                    # t-block for scores pass
NTB = T // TB               # 8
NPAIR = H // 2              # 8 head-pairs
F32 = mybir.dt.float32
F16 = mybir.dt.float16
import os
N_XBAR_ZC = int(os.environ.get("N_XBAR_ZC", "4"))  # z-chunks via XBAR DMA-transpose


def build_nc():
    nc = bacc.Bacc(None, target_bir_lowering=False)

    o16 = nc.declare_dram_parameter("o16", [BLOC, T, Z], F16, isOutput=False)
    o_lastT = nc.declare_dram_parameter("o_lastT", [P, ZC, BLOC], F16, isOutput=False)
    wq16 = nc.declare_dram_parameter("Wq16", [P, ZC, Z], F16, isOutput=False)
    wkT16 = nc.declare_dram_parameter("WkT16", [P, NPAIR, Z], F16, isOutput=False)
    wv16 = nc.declare_dram_parameter("Wv16", [P, ZC, Z], F16, isOutput=False)
    bq_r = nc.declare_dram_parameter("bq_r", [P, ZC], F32, isOutput=False)
    bv_in = nc.declare_dram_parameter("bv", [H, DK], F32, isOutput=False)
    dmask = nc.declare_dram_parameter("dmask", [H, Z], F32, isOutput=False)
    out = nc.declare_dram_parameter("out", [BLOC, Z], F32, isOutput=True)

    with tile.TileContext(nc) as tc:
        with (
            tc.tile_pool(name="const", bufs=1) as const,
            tc.tile_pool(name="small", bufs=2) as small,
        ):
            ident = const.tile([P, P], F16)
            make_identity(nc, ident)
            identf = const.tile([P, P], F32)
            make_identity(nc, identf)
            bv_sb = const.tile([H, DK], F32)
            nc.sync.dma_start(out=bv_sb, in_=bv_in[:])
            bqr_sb = const.tile([P, ZC], F32)
            nc.sync.dma_start(out=bqr_sb, in_=bq_r[:])
            dmask_sb = const.tile([H, Z], F32)
            nc.sync.dma_start(out=dmask_sb, in_=dmask[:])

            wkq_sb = []  # per-batch (P, ZC, H) fp16
            # ---------------- prologue: q and wkq for both batches ----------
            with (
                tc.tile_pool(name="wpro", bufs=1) as wpro,
                tc.tile_pool(name="propsum", bufs=2, space="PSUM") as propsum,
            ):
                wq_sb = wpro.tile([P, ZC, Z], F16)   # [zp, zc, m]
                for zc in range(ZC):
                    nc.sync.dma_start(out=wq_sb[:, zc, :], in_=wq16[:, zc, :])
                wkT_sb = wpro.tile([P, NPAIR, Z], F16)  # [dd, pair, z]
                for pr in range(NPAIR):
                    nc.sync.dma_start(out=wkT_sb[:, pr, :], in_=wkT16[:, pr, :])
                olT_sb = wpro.tile([P, ZC, BLOC], F16)
                nc.sync.dma_start(out=olT_sb, in_=o_lastT[:])

                # q (full vector of H*DK=Z, chunked 128): q_sb[mp, mc, b] fp32
                q_sb = wpro.tile([P, ZC, BLOC], F32)
                for mc in range(ZC):
                    qp = propsum.tile([P, BLOC], F32, tag="qp")
                    for zc in range(ZC):
                        nc.tensor.matmul(
                            qp,
                            wq_sb[:, zc, mc * P : (mc + 1) * P],
                            olT_sb[:, zc, :],
                            start=(zc == 0),
                            stop=(zc == ZC - 1),
                        )
                    nc.vector.tensor_tensor(
                        q_sb[:, mc, :],
                        qp,
                        bqr_sb[:, mc : mc + 1].to_broadcast((P, BLOC)),
                        mybir.AluOpType.add,
                    )

                for b in range(BLOC):
                    qsel = wpro.tile([P, NPAIR, 2], F16, tag=f"qsel{b}")
                    nc.vector.memset(qsel, 0.0)
                    for pr in range(NPAIR):
                        nc.vector.tensor_copy(
                            out=qsel[0:DK, pr, 0:1], in_=q_sb[0:DK, pr, b : b + 1]
                        )
                        nc.vector.tensor_copy(
                            out=qsel[DK:P, pr, 1:2], in_=q_sb[DK:P, pr, b : b + 1]
                        )
                    wkq_b = const.tile([P, ZC, H], F16, tag=f"wkq{b}")
                    for zc in range(ZC):
                        wp = propsum.tile([P, H], F32, tag="wp")
                        for pr in range(NPAIR):
                            nc.tensor.matmul(
                                wp[:, 2 * pr : 2 * pr + 2],
                                wkT_sb[:, pr, zc * P : (zc + 1) * P],
                                qsel[:, pr, :],
                                start=True,
                                stop=True,
                            )
                        nc.any.tensor_copy(out=wkq_b[:, zc, :], in_=wp)
                    wkq_sb.append(wkq_b)

            # ---------------- main per-batch pipeline -----------------------
            with (
                tc.tile_pool(name="abuf", bufs=1) as abuf,
                tc.tile_pool(name="wvp", bufs=1) as wvp,
                tc.tile_pool(name="stage", bufs=3) as stage,
                tc.tile_pool(name="xstage", bufs=8) as xstage,
                tc.tile_pool(name="tpsum", bufs=2, space="PSUM") as tpsum,
                tc.tile_pool(name="mpsum", bufs=2, space="PSUM") as mpsum,
                tc.tile_pool(name="rpsum", bufs=1, space="PSUM") as rpsum,
            ):
                wv_sb = wvp.tile([P, ZC, Z], F16)  # [zp, zc, h*64+d]
                for zc in range(ZC):
                    nc.sync.dma_start(out=wv_sb[:, zc, :], in_=wv16[:, zc, :])

                # per-batch zero-padded transpose staging (rows >= H stay 0)
                pT_pads, r_pads = [], []
                for i in range(BLOC):
                    tpad = wvp.tile([P, T], F32, tag=f"pTp{i}")
                    nc.vector.memset(tpad[:, :], 0.0)
                    pT_pads.append(tpad)
                    rp = wvp.tile([P, Z], F32, tag=f"rp{i}")
                    nc.vector.memset(rp[:, :], 0.0)
                    r_pads.append(rp)

                for b in range(BLOC):
                    # A (fp16) as 8 blocks of (P, 4, Z); t = blk*512 + i*128 + p
                    a_sb = []
                    for blk in range(8):
                        a_t = abuf.tile([P, 4, Z], F16, tag=f"a{blk}")
                        nc.sync.dma_start(
                            out=a_t,
                            in_=o16[b, blk * 512 : (blk + 1) * 512, :].rearrange(
                                "(i zp) z -> zp i z", zp=P
                            ),
                        )
                        a_sb.append(a_t)

                    # scores^T (H, T) in fp32, staged per batch
                    pT_pad = pT_pads[b]

                    for tb in range(NTB):
                        sc_ps = mpsum.tile([H, TB], F32, tag="scps")
                        # XBAR-supplied A^T tiles for zc in [0, N_XBAR_ZC)
                        for zc in range(N_XBAR_ZC):
                            atx = xstage.tile([P, TB], F16, tag="atx")
                            nc.sync.dma_start_transpose(
                                atx,
                                o16[b, tb * TB : (tb + 1) * TB,
                                    zc * P : (zc + 1) * P],
                            )
                            nc.tensor.matmul(
                                sc_ps,
                                wkq_sb[b][:, zc, :],
                                atx[:],
                                start=(zc == 0),
                                stop=False,
                            )
                        # PE-transposed A^T tiles for the rest, 8 per PSUM bank
                        for g in range((ZC - N_XBAR_ZC) // 2):
                            zc0 = N_XBAR_ZC + 2 * g
                            at_ps = tpsum.tile([P, 2 * TB], F16, tag="atps")
                            for j in range(2):
                                for i in range(4):
                                    nc.tensor.transpose(
                                        at_ps[:, j * TB + i * P : j * TB + (i + 1) * P],
                                        a_sb[tb][:, i, (zc0 + j) * P : (zc0 + j + 1) * P],
                                        ident,
                                    )
                            at16 = stage.tile([P, 2 * TB], F16, tag="at16")
                            if g % 2 == 0:
                                nc.vector.tensor_copy(out=at16, in_=at_ps)
                            else:
                                nc.scalar.copy(out=at16, in_=at_ps)
                            for j in range(2):
                                zc = zc0 + j
                                nc.tensor.matmul(
                                    sc_ps,
                                    wkq_sb[b][:, zc, :],
                                    at16[:, j * TB : (j + 1) * TB],
                                    start=False,
                                    stop=(zc == ZC - 1),
                                )
                        nc.any.tensor_copy(
                            out=pT_pad[:H, tb * TB : (tb + 1) * TB], in_=sc_ps
                        )

                    # softmax rows 0..H-1 in place: p^T = exp(s^T - max)
                    mx = small.tile([H, 1], F32, tag="mx")
                    nc.vector.reduce_max(mx, pT_pad[:H, :], axis=mybir.AxisListType.X)
                    negmax = small.tile([H, 1], F32, tag="negmax")
                    nc.scalar.mul(out=negmax, in_=mx, mul=-1.0)
                    lsum = small.tile([H, 1], F32, tag="lsum")
                    nc.scalar.activation(
                        out=pT_pad[:H, :],
                        in_=pT_pad[:H, :],
                        func=mybir.ActivationFunctionType.Exp,
                        bias=negmax,
                        scale=1.0,
                        accum_out=lsum,
                    )
                    rinv = small.tile([H, 1], F32, tag="rinv")
                    nc.vector.reciprocal(rinv, lsum)

                    # p natural (t on partitions), fp16; 4 transposes per bank
                    p_sb = stage.tile([P, NT, H], F16, tag="psb")
                    for g in range(NT // 4):
                        pp = tpsum.tile([P, 4, P], F32, tag="tp")
                        for i in range(4):
                            tt = g * 4 + i
                            nc.tensor.transpose(
                                pp[:, i, :], pT_pad[:, tt * P : (tt + 1) * P], identf
                            )
                        if g % 2 == 0:
                            nc.vector.tensor_copy(
                                out=p_sb[:, g * 4 : (g + 1) * 4, :], in_=pp[:, :, :H]
                            )
                        else:
                            nc.scalar.copy(
                                out=p_sb[:, g * 4 : (g + 1) * 4, :], in_=pp[:, :, :H]
                            )

                    # r (H, Z) = p^T A accumulated over t (fp32 psum)
                    r_ps = rpsum.tile([H, 2, TB], F32, tag="rps")
                    for tt in range(NT):
                        blk, i = tt // 4, tt % 4
                        for zt in range(2):
                            nc.tensor.matmul(
                                r_ps[:, zt, :],
                                p_sb[:, tt, :],
                                a_sb[blk][:, i, zt * TB : (zt + 1) * TB],
                                start=(tt == 0),
                                stop=(tt == NT - 1),
                            )
                    r_pad = r_pads[b]
                    nc.any.tensor_copy(
                        out=r_pad[:H, :], in_=r_ps.rearrange("h a f -> h (a f)")
                    )

                    # r^T chunks (z on partitions) fp16: rt_sb[zp, zc, h]
                    rt_sb = stage.tile([P, ZC, H], F16, tag="rtsb")
                    for g in range(2):
                        rt_ps = tpsum.tile([P, 4, P], F32, tag="tp")
                        for i in range(4):
                            zc = g * 4 + i
                            nc.tensor.transpose(
                                rt_ps[:, i, :], r_pad[:, zc * P : (zc + 1) * P], identf
                            )
                        nc.any.tensor_copy(
                            out=rt_sb[:, g * 4 : (g + 1) * 4, :], in_=rt_ps[:, :, :H]
                        )

                    # ctx_full[h', m] = sum_z r[h',z] WvF[z, m]; diag blocks kept
                    cf_ps = rpsum.tile([H, 2, TB], F32, tag="rps")
                    for mt in range(2):
                        for zc in range(ZC):
                            nc.tensor.matmul(
                                cf_ps[:, mt, :],
                                rt_sb[:, zc, :],
                                wv_sb[:, zc, mt * TB : (mt + 1) * TB],
                                start=(zc == 0),
                                stop=(zc == ZC - 1),
                            )
                    masked = small.tile([H, Z], F32, tag="masked")
                    nc.vector.tensor_tensor(
                        masked,
                        cf_ps.rearrange("h a f -> h (a f)"),
                        dmask_sb,
                        mybir.AluOpType.mult,
                    )
                    ctx_sb = small.tile([H, DK], F32, tag="ctxsb")
                    nc.vector.reduce_sum(
                        ctx_sb,
                        masked.rearrange("h (g d) -> h d g", d=DK),
                        axis=mybir.AxisListType.X,
                    )

                    out_sb = small.tile([H, DK], F32, tag="outsb")
                    nc.vector.tensor_scalar_mul(
                        out=out_sb, in0=ctx_sb, scalar1=rinv
                    )
                    nc.vector.tensor_add(out=out_sb, in0=out_sb, in1=bv_sb)
                    nc.sync.dma_start(
                        out=out[b].rearrange("(h d) -> h d", h=H), in_=out_sb
                    )

    nc.finalize()
    return nc


_NC_CACHE = {}


def _get_nc():
    if "nc" not in _NC_CACHE:
        _NC_CACHE["nc"] = build_nc()
    return _NC_CACHE["nc"]


def prep_inputs(o_all, o_last, Wk, Wv, Wq, bk, bv, bq):
    """Host-side shard + layout prep. Returns per-core input maps."""
    o_all = np.asarray(o_all, dtype=np.float32)
    o_last = np.asarray(o_last, dtype=np.float32)
    Wk = np.asarray(Wk, dtype=np.float32)
    Wv = np.asarray(Wv, dtype=np.float32)
    Wq = np.asarray(Wq, dtype=np.float32)
    bv = np.asarray(bv, dtype=np.float32)
    bq = np.asarray(bq, dtype=np.float32)

    wq_flat = Wq.transpose(1, 0, 2).reshape(Z, Z)
    wq16 = np.ascontiguousarray(
        wq_flat.reshape(ZC, P, Z).transpose(1, 0, 2)
    ).astype(np.float16)
    wkT16 = np.ascontiguousarray(
        Wk.transpose(0, 2, 1).reshape(NPAIR, P, Z).transpose(1, 0, 2)
    ).astype(np.float16)
    wv_flat = Wv.transpose(1, 0, 2).reshape(Z, Z)
    wv16 = np.ascontiguousarray(
        wv_flat.reshape(ZC, P, Z).transpose(1, 0, 2)
    ).astype(np.float16)
    bq_r = np.ascontiguousarray(bq.reshape(Z).reshape(ZC, P).T)  # [P, ZC]
    bv_c = np.ascontiguousarray(bv)
    dmask = np.zeros((H, Z), dtype=np.float32)
    for h in range(H):
        dmask[h, h * DK : (h + 1) * DK] = 1.0

    in_maps = []
    for c in range(NCORES):
        sl = slice(c * BLOC, (c + 1) * BLOC)
        olT16 = np.ascontiguousarray(
            o_last[sl, 0, :].T.reshape(ZC, P, BLOC).transpose(1, 0, 2)
        ).astype(np.float16)
        in_maps.append(
            {
                "o16": o_all[sl].astype(np.float16),
                "o_lastT": olT16,
                "Wq16": wq16,
                "WkT16": wkT16,
                "Wv16": wv16,
                "bq_r": bq_r,
                "bv": bv_c,
                "dmask": dmask,
            }
        )
    return in_maps


def kernel(o_all, o_last, Wk, Wv, Wq, bk, bv, bq, _trace=False, _trace_kwargs=None):
    nc = _get_nc()
    in_maps = prep_inputs(o_all, o_last, Wk, Wv, Wq, bk, bv, bq)
    res = run_bass_kernel_spmd(
        nc, in_maps, core_ids=list(range(NCORES)), trace=_trace,
        **(_trace_kwargs or {}),
    )
    outs = [r["out"] for r in res.results]
    full = np.concatenate(outs, axis=0).reshape(B, 1, Z)
    if _trace:
        kernel.last_result = res
    return full



# revision 8
# speedup vs baseline: 2.0477x; 2.0477x over previous
"""MultiHeadTimeDimensionAttention kernel for Trainium2 (8 NeuronCores).

Math (per batch b):
  q[h,d]      = o_last[b] . Wq[h,:,d] + bq[h,d]          (host, 0.4% of FLOPs)
  wkq[z,h]    = sum_d Wk[h,z,d] q[h,d]                   (host)
  scores[t,h] = sum_z o_all[b,t,z] * wkq[z,h]            (device, bk drops: softmax-invariant)
  p = exp(scores - max_t), L = sum_t p
  r[h,z]      = sum_t p[t,h] * o_all[b,t,z]
  ctx[h,d]    = (sum_z r[h,z] Wv[h,z,d]) / L[h] + bv[h,d]

Device layout: scores^T kept in a (tb,h)-packed [128, 512] PSUM layout via
column-tiled (tile_position) M=16 matmuls, 4 t-blocks concurrent on the PE
array.  Softmax runs at full 128-partition parallelism; cross-partition
head reductions go through tiny PE transposes + a K=1 broadcast matmul.
A^T tiles are produced on-PE from the (single) natural-layout copy of
o_all; fp16 everywhere on the PE, fp32 PSUM/softmax.

Sharding: data-parallel over B; each core handles B/8 = 2 batches.
"""

import numpy as np

import concourse.bacc as bacc
import concourse.tile as tile
import concourse.mybir as mybir
from concourse.bass_utils import run_bass_kernel_spmd
from concourse.masks import make_identity

B, T, Z, H, DK = 16, 4096, 1024, 16, 64
P = 128
NCORES = 8
BLOC = B // NCORES          # batches per core
ZC = Z // P                 # 8 z-chunks
TB = 512                    # t-block (one PSUM bank column span)
NTBG = 2                    # two groups of 4 t-blocks per batch
F32 = mybir.dt.float32
F16 = mybir.dt.float16
EXP = mybir.ActivationFunctionType.Exp
AX = mybir.AxisListType.X
MULT = mybir.AluOpType.mult


def build_nc():
    nc = bacc.Bacc(None, target_bir_lowering=False)

    o16 = nc.declare_dram_parameter("o16", [BLOC, T, Z], F16, isOutput=False)
    wv16 = nc.declare_dram_parameter("Wv16", [P, ZC, Z], F16, isOutput=False)
    wkq16 = nc.declare_dram_parameter("wkq16", [BLOC, P, ZC, H], F16, isOutput=False)
    bv128 = nc.declare_dram_parameter("bv128", [P, DK], F32, isOutput=False)
    dmask = nc.declare_dram_parameter("dmask", [P, 256], F32, isOutput=False)
    out = nc.declare_dram_parameter("out", [BLOC, Z], F32, isOutput=True)

    with tile.TileContext(nc) as tc:
        with (
            tc.tile_pool(name="const", bufs=1) as const,
            tc.tile_pool(name="abuf", bufs=2) as abuf,
            tc.tile_pool(name="atbuf", bufs=1) as atbuf,
            tc.tile_pool(name="stage", bufs=2) as stage,
            tc.tile_pool(name="small", bufs=2) as small,
            tc.tile_pool(name="scp", bufs=2, space="PSUM") as scp,
            tc.tile_pool(name="atp", bufs=2, space="PSUM") as atp,
            tc.tile_pool(name="xps", bufs=1, space="PSUM") as xps,
            tc.tile_pool(name="rp", bufs=1, space="PSUM") as rp,
            tc.tile_pool(name="cfp", bufs=1, space="PSUM") as cfp,
        ):
            # ---------------- constants (weights go on the Act DMA ring) ----
            ident16 = const.tile([P, P], F16)
            make_identity(nc, ident16)
            identf = const.tile([P, P], F32)
            make_identity(nc, identf)
            onesf = const.tile([1, 1], F32)
            nc.vector.memset(onesf, 1.0)

            wv_sb = const.tile([P, ZC, Z], F16)
            wkq_sb = []
            for b in range(BLOC):
                wkq_b = const.tile([P, ZC, H], F16, tag=f"wkq{b}")
                nc.scalar.dma_start(out=wkq_b, in_=wkq16[b])
                wkq_sb.append(wkq_b)
            for zc in range(ZC):
                nc.scalar.dma_start(out=wv_sb[:, zc, :], in_=wv16[:, zc, :])
            bv_sb = const.tile([P, DK], F32)
            nc.scalar.dma_start(out=bv_sb, in_=bv128[:])
            dmask_sb = const.tile([P, 256], F32)
            nc.scalar.dma_start(out=dmask_sb, in_=dmask[:])

            for b in range(BLOC):
                # ---- A natural layout, 8 × 1MB loads on the sync ring ----
                a_sb = abuf.tile([P, 32, Z], F16, tag="a")
                for blk in range(8):
                    nc.sync.dma_start(
                        out=a_sb[:, blk * 4 : (blk + 1) * 4, :],
                        in_=o16[b, blk * 512 : (blk + 1) * 512, :].rearrange(
                            "(i zp) z -> zp i z", zp=P
                        ),
                    )

                # ---- scores: per t-block-group, A^T on PE + col-tiled mms --
                sc_tiles = []
                p_sb = stage.tile([P, NTBG, TB], F16, tag="p")
                for tbg in range(NTBG):
                    at_sb = atbuf.tile([P, 4, ZC, TB], F16, tag="at")
                    nfill = 0
                    for j in range(4):          # strip j <-> t-block tbg*4+j
                        for zcp in range(4):    # pairs of z-chunks
                            at_ps = atp.tile([P, 8, P], F16, tag="atp")
                            for zz in range(2):
                                zc = 2 * zcp + zz
                                for i in range(4):
                                    gi = (tbg * 4 + j) * 4 + i
                                    nc.tensor.transpose(
                                        at_ps[:, 4 * zz + i, :],
                                        a_sb[:, gi, zc * P : (zc + 1) * P],
                                        ident16,
                                    )
                            eng = nc.vector if nfill % 2 == 0 else nc.scalar
                            cp = eng.tensor_copy if nfill % 2 == 0 else eng.copy
                            cp(
                                out=at_sb[:, j, 2 * zcp : 2 * zcp + 2, :],
                                in_=at_ps.rearrange(
                                    "p (zz i) c -> p zz (i c)", zz=2
                                ),
                            )
                            nfill += 1
                    sc_ps = scp.tile([P, TB], F32, tag="sc")
                    for zc in range(ZC):
                        for j in range(4):
                            nc.tensor.matmul(
                                sc_ps[32 * j : 32 * j + 16, :],
                                wkq_sb[b][:, zc, :],
                                at_sb[:, j, zc, :],
                                start=(zc == 0),
                                stop=(zc == ZC - 1),
                                tile_position=(0, 32 * j),
                            )
                    sc_tiles.append(sc_ps)

                # ---- softmax over t at 128-partition parallelism ----------
                # partition layout: p = 32*(tb%4) + h, bank = tb//4
                m_sb = small.tile([P, 2], F32, tag="m")
                for tbg in range(NTBG):
                    nc.vector.reduce_max(
                        m_sb[:, tbg : tbg + 1], sc_tiles[tbg], axis=AX
                    )
                mm1 = small.tile([P, 1], F32, tag="mm1")
                nc.vector.reduce_max(mm1, m_sb, axis=AX)
                xs = xps.tile([P, P], F32, tag="xs")
                nc.tensor.transpose(xs[0:1, :], mm1, identf)
                mTs = small.tile([1, P], F32, tag="mTs")
                nc.vector.tensor_copy(mTs, xs[0:1, :])
                M32 = small.tile([1, 32], F32, tag="M32")
                nc.vector.reduce_max(
                    M32, mTs.rearrange("a (j c) -> a c j", j=4), axis=AX
                )
                negM = small.tile([1, 32], F32, tag="negM")
                nc.scalar.mul(out=negM, in_=M32, mul=-1.0)
                negMr = small.tile([1, 4, 32], F32, tag="negMr")
                nc.vector.tensor_copy(
                    negMr, negM.unsqueeze(1).to_broadcast((1, 4, 32))
                )
                xs = xps.tile([P, P], F32, tag="xs")
                nc.tensor.matmul(
                    xs[:, 0:1], negMr, onesf, start=True, stop=True
                )
                negM128 = small.tile([P, 1], F32, tag="negM128")
                nc.vector.tensor_copy(negM128, xs[:, 0:1])

                ls_sb = small.tile([P, 2], F32, tag="ls")
                for tbg in range(NTBG):
                    nc.scalar.activation(
                        out=p_sb[:, tbg, :],
                        in_=sc_tiles[tbg],
                        func=EXP,
                        bias=negM128,
                        scale=1.0,
                        accum_out=ls_sb[:, tbg : tbg + 1],
                    )
                ls1 = small.tile([P, 1], F32, tag="ls1")
                nc.vector.reduce_sum(ls1, ls_sb, axis=AX)
                xs = xps.tile([P, P], F32, tag="xs")
                nc.tensor.transpose(xs[0:1, :], ls1, identf)
                lTs = small.tile([1, P], F32, tag="lTs")
                nc.vector.tensor_copy(lTs, xs[0:1, :])
                L32 = small.tile([1, 32], F32, tag="L32")
                nc.vector.reduce_sum(
                    L32, lTs.rearrange("a (j c) -> a c j", j=4), axis=AX
                )
                rinv32 = small.tile([1, 32], F32, tag="rinv32")
                nc.vector.reciprocal(rinv32, L32)
                rinvr = small.tile([1, 4, 32], F32, tag="rinvr")
                nc.vector.tensor_copy(
                    rinvr, rinv32.unsqueeze(1).to_broadcast((1, 4, 32))
                )
                xs = xps.tile([P, P], F32, tag="xs")
                nc.tensor.matmul(
                    xs[:, 0:1], rinvr, onesf, start=True, stop=True
                )
                rinv128 = small.tile([P, 1], F32, tag="rinv128")
                nc.vector.tensor_copy(rinv128, xs[:, 0:1])

                # ---- p natural (t on partitions) via PE transposes --------
                ptT = []
                for tbg in range(NTBG):
                    pt_ps = xps.tile([P, 4, P], F16, tag="ptT")
                    for i in range(4):
                        nc.tensor.transpose(
                            pt_ps[:, i, :],
                            p_sb[:, tbg, i * P : (i + 1) * P],
                            ident16,
                        )
                    pt_sb = stage.tile([P, 4, P], F16, tag=f"ptT{tbg}")
                    nc.vector.tensor_copy(pt_sb, pt_ps)
                    ptT.append(pt_sb)

                # ---- r[h, z] col-tiled over z-quarters --------------------
                r_ps = rp.tile([P, 256], F32, tag="r")
                nmm = 0
                for tbg in range(NTBG):
                    for i in range(4):
                        for jt in range(4):
                            gi = (tbg * 4 + jt) * 4 + i
                            for j in range(4):
                                nc.tensor.matmul(
                                    r_ps[32 * j : 32 * j + 16, :],
                                    ptT[tbg][:, i, 32 * jt : 32 * jt + 16],
                                    a_sb[:, gi, j * 256 : (j + 1) * 256],
                                    start=(nmm == 0),
                                    stop=(nmm == 31),
                                    tile_position=(0, 32 * j),
                                )
                            nmm += 1

                r16 = stage.tile([P, 256], F16, tag="r16")
                nc.vector.tensor_copy(r16, r_ps)
                rT_ps = xps.tile([P, 4, P], F16, tag="ptT")
                for half in range(2):
                    nc.tensor.transpose(
                        rT_ps[:, half, :],
                        r16[:, half * P : (half + 1) * P],
                        ident16,
                    )
                rt_sb = stage.tile([P, 2, P], F16, tag="rt")
                nc.vector.tensor_copy(rt_sb, rT_ps[:, 0:2, :])

                # ---- ctx_full = r @ Wv, col-tiled over m-quarters ---------
                cf_ps = cfp.tile([P, 256], F32, tag="cf")
                for zc in range(ZC):
                    half, zq = zc % 2, zc // 2
                    for j in range(4):
                        nc.tensor.matmul(
                            cf_ps[32 * j : 32 * j + 16, :],
                            rt_sb[:, half, 32 * zq : 32 * zq + 16],
                            wv_sb[:, zc, j * 256 : (j + 1) * 256],
                            start=(zc == 0),
                            stop=(zc == ZC - 1),
                            tile_position=(0, 32 * j),
                        )

                # ---- diag extract, normalize, bias, store -----------------
                ctxm = stage.tile([P, 256], F32, tag="ctxm")
                nc.vector.tensor_tensor(ctxm, cf_ps, dmask_sb, MULT)
                ctxr = stage.tile([P, DK], F32, tag="ctxr")
                nc.vector.reduce_sum(
                    ctxr, ctxm.rearrange("p (g d) -> p d g", d=DK), axis=AX
                )
                ctxs = stage.tile([P, DK], F32, tag="ctxs")
                nc.vector.tensor_scalar_mul(out=ctxs, in0=ctxr, scalar1=rinv128)
                nc.vector.tensor_add(out=ctxs, in0=ctxs, in1=bv_sb)
                outv = out[b].rearrange("(h d) -> h d", h=H)
                for j in range(4):
                    nc.sync.dma_start(
                        out=outv[4 * j : 4 * j + 4, :],
                        in_=ctxs[36 * j : 36 * j + 4, :],
                    )

    nc.finalize()
    return nc


_NC_CACHE = {}


def _get_nc():
    if "nc" not in _NC_CACHE:
        _NC_CACHE["nc"] = build_nc()
    return _NC_CACHE["nc"]


def prep_inputs(o_all, o_last, Wk, Wv, Wq, bk, bv, bq):
    """Host-side shard + layout prep. Returns per-core input maps."""
    o_all = np.asarray(o_all, dtype=np.float32)
    o_last = np.asarray(o_last, dtype=np.float32)
    Wk = np.asarray(Wk, dtype=np.float32)
    Wv = np.asarray(Wv, dtype=np.float32)
    Wq = np.asarray(Wq, dtype=np.float32)
    bv = np.asarray(bv, dtype=np.float32)
    bq = np.asarray(bq, dtype=np.float32)

    # q for all batches, then wkq[z, h] = sum_d Wk[h,z,d] q[h,d]
    wq_flat = Wq.transpose(1, 0, 2).reshape(Z, Z)
    q_all = o_last[:, 0, :] @ wq_flat + bq.reshape(Z)          # [B, Z]
    wkq_all = np.einsum(
        "hzd,bhd->bzh", Wk, q_all.reshape(B, H, DK), optimize=True
    )                                                           # [B, Z, H]

    wv_flat = Wv.transpose(1, 0, 2).reshape(Z, Z)
    wv16 = np.ascontiguousarray(
        wv_flat.reshape(ZC, P, Z).transpose(1, 0, 2)
    ).astype(np.float16)

    bv128 = np.zeros((P, DK), dtype=np.float32)
    dmask = np.zeros((P, 256), dtype=np.float32)
    for h in range(H):
        j, r = h // 4, h % 4
        bv128[36 * j + r] = bv[h]
        dmask[32 * j + h, DK * r : DK * (r + 1)] = 1.0

    in_maps = []
    for c in range(NCORES):
        sl = slice(c * BLOC, (c + 1) * BLOC)
        wkq16 = np.ascontiguousarray(
            wkq_all[sl].reshape(BLOC, ZC, P, H).transpose(0, 2, 1, 3)
        ).astype(np.float16)
        in_maps.append(
            {
                "o16": o_all[sl].astype(np.float16),
                "Wv16": wv16,
                "wkq16": wkq16,
                "bv128": bv128,
                "dmask": dmask,
            }
        )
    return in_maps


def kernel(o_all, o_last, Wk, Wv, Wq, bk, bv, bq, _trace=False, _trace_kwargs=None):
    nc = _get_nc()
    in_maps = prep_inputs(o_all, o_last, Wk, Wv, Wq, bk, bv, bq)
    res = run_bass_kernel_spmd(
        nc, in_maps, core_ids=list(range(NCORES)), trace=_trace,
        **(_trace_kwargs or {}),
    )
    outs = [r["out"] for r in res.results]
    full = np.concatenate(outs, axis=0).reshape(B, 1, Z)
    if _trace:
        kernel.last_result = res
    return full


# revision 10
# speedup vs baseline: 2.1920x; 1.0705x over previous
"""MultiHeadTimeDimensionAttention kernel for Trainium2 (8 NeuronCores).

Math (per batch b):
  q[h,d]      = o_last[b] . Wq[h,:,d] + bq[h,d]          (host, 0.4% of FLOPs)
  wkq[z,h]    = sum_d Wk[h,z,d] q[h,d]                   (host)
  scores[t,h] = sum_z o_all[b,t,z] * wkq[z,h]            (device, bk drops: softmax-invariant)
  p = exp(scores - max_t), L = sum_t p
  r[h,z]      = sum_t p[t,h] * o_all[b,t,z]
  ctx[h,d]    = (sum_z r[h,z] Wv[h,z,d]) / L[h] + bv[h,d]

Device layout: scores^T kept in a (tb,h)-packed [128, 512] PSUM layout via
column-tiled (tile_position) M=16 matmuls, 4 t-blocks concurrent on the PE
array.  Softmax runs at full 128-partition parallelism; cross-partition
head reductions go through tiny PE transposes + a K=1 broadcast matmul.
A^T tiles are produced on-PE from the (single) natural-layout copy of
o_all; fp16 everywhere on the PE, fp32 PSUM/softmax.

The two batches per core are software-pipelined: batch 1's transpose fills
and score matmuls are emitted between batch 0's stages so the PE stays busy
during batch 0's softmax and the DMA stream stays ahead of compute.

Sharding: data-parallel over B; each core handles B/8 = 2 batches.
"""

import numpy as np

import concourse.bacc as bacc
import concourse.tile as tile
import concourse.mybir as mybir
from concourse.bass_utils import run_bass_kernel_spmd
from concourse.masks import make_identity

B, T, Z, H, DK = 16, 4096, 1024, 16, 64
P = 128
NCORES = 8
BLOC = B // NCORES          # batches per core
ZC = Z // P                 # 8 z-chunks
TB = 512                    # t-block (one PSUM bank column span)
NTBG = 2                    # two groups of 4 t-blocks per batch
F32 = mybir.dt.float32
F16 = mybir.dt.float16
EXP = mybir.ActivationFunctionType.Exp
AX = mybir.AxisListType.X
MULT = mybir.AluOpType.mult


def build_nc():
    nc = bacc.Bacc(None, target_bir_lowering=False)

    o16 = nc.declare_dram_parameter("o16", [BLOC, T, Z], F16, isOutput=False)
    wv16 = nc.declare_dram_parameter("Wv16", [P, ZC, Z], F16, isOutput=False)
    wkq16 = nc.declare_dram_parameter("wkq16", [BLOC, P, ZC, H], F16, isOutput=False)
    bv128 = nc.declare_dram_parameter("bv128", [P, DK], F32, isOutput=False)
    dmask = nc.declare_dram_parameter("dmask", [P, 256], F32, isOutput=False)
    out = nc.declare_dram_parameter("out", [BLOC, Z], F32, isOutput=True)

    with tile.TileContext(nc) as tc:
        with (
            tc.tile_pool(name="const", bufs=1) as const,
            tc.tile_pool(name="abuf", bufs=2) as abuf,
            tc.tile_pool(name="atbuf", bufs=1) as atbuf,
            tc.tile_pool(name="stage", bufs=2) as stage,
            tc.tile_pool(name="small", bufs=2) as small,
            tc.tile_pool(name="scp", bufs=2, space="PSUM") as scp,
            tc.tile_pool(name="atp", bufs=2, space="PSUM") as atp,
            tc.tile_pool(name="xps", bufs=1, space="PSUM") as xps,
            tc.tile_pool(name="rp", bufs=1, space="PSUM") as rp,
            tc.tile_pool(name="cfp", bufs=1, space="PSUM") as cfp,
        ):
            ident16 = const.tile([P, P], F16)
            make_identity(nc, ident16)
            identf = const.tile([P, P], F32)
            make_identity(nc, identf)
            onesf = const.tile([1, 1], F32)
            nc.vector.memset(onesf, 1.0)

            # ------------- DMA schedule (single sync ring, FIFO starts) -----
            # tiny first, then b0 stream, then b1 stream with wv interleaved
            wkq_sb = []
            for b in range(BLOC):
                wkq_b = const.tile([P, ZC, H], F16, tag=f"wkq{b}")
                nc.sync.dma_start(out=wkq_b, in_=wkq16[b])
                wkq_sb.append(wkq_b)
            dmask_sb = const.tile([P, 256], F32)
            nc.sync.dma_start(out=dmask_sb, in_=dmask[:])
            bv_sb = const.tile([P, DK], F32)
            nc.sync.dma_start(out=bv_sb, in_=bv128[:])

            wv_sb = const.tile([P, ZC, Z], F16)
            a_sbs = []

            def load_blocks(b, blks):
                for blk in blks:
                    nc.sync.dma_start(
                        out=a_sbs[b][:, blk * 4 : (blk + 1) * 4, :],
                        in_=o16[b, blk * 512 : (blk + 1) * 512, :].rearrange(
                            "(i zp) z -> zp i z", zp=P
                        ),
                    )

            for b in range(BLOC):
                a_sb = abuf.tile([P, 32, Z], F16, tag="a", name=f"a_sb{b}")
                a_sbs.append(a_sb)
            load_blocks(0, range(8))
            load_blocks(1, range(0, 3))
            for zc in range(4):
                nc.sync.dma_start(out=wv_sb[:, zc, :], in_=wv16[:, zc, :])
            load_blocks(1, range(3, 6))
            for zc in range(4, 8):
                nc.sync.dma_start(out=wv_sb[:, zc, :], in_=wv16[:, zc, :])
            load_blocks(1, range(6, 8))

            # ------------- stage emitters ----------------------------------
            def emit_fills(b, tbg):
                """A^T tiles for t-blocks tbg*4..tbg*4+3 via PE transposes."""
                at_sb = atbuf.tile([P, 4, ZC, TB], F16, tag="at")
                nfill = 0
                for j in range(4):
                    for zcp in range(4):
                        at_ps = atp.tile([P, 8, P], F16, tag="atp")
                        for zz in range(2):
                            zc = 2 * zcp + zz
                            for i in range(4):
                                gi = (tbg * 4 + j) * 4 + i
                                nc.tensor.transpose(
                                    at_ps[:, 4 * zz + i, :],
                                    a_sbs[b][:, gi, zc * P : (zc + 1) * P],
                                    ident16,
                                )
                        eng = nc.scalar if nfill % 3 == 2 else nc.vector
                        cp = eng.copy if nfill % 3 == 2 else eng.tensor_copy
                        cp(
                            out=at_sb[:, j, 2 * zcp : 2 * zcp + 2, :],
                            in_=at_ps.rearrange("p (zz i) c -> p zz (i c)", zz=2),
                        )
                        nfill += 1
                return at_sb

            def emit_scores(b, tbg, at_sb):
                sc_ps = scp.tile([P, TB], F32, tag="sc")
                for zc in range(ZC):
                    for j in range(4):
                        nc.tensor.matmul(
                            sc_ps[32 * j : 32 * j + 16, :],
                            wkq_sb[b][:, zc, :],
                            at_sb[:, j, zc, :],
                            start=(zc == 0),
                            stop=(zc == ZC - 1),
                            tile_position=(0, 32 * j),
                        )
                return sc_ps

            def emit_softmax(b, sc_tiles):
                """softmax over t; scores live as [128=(tb%4)*32+h, 512]x2."""
                p_sb = stage.tile([P, NTBG, TB], F16, tag="p")
                m_sb = small.tile([P, 2], F32, tag="m")
                for tbg in range(NTBG):
                    nc.vector.reduce_max(
                        m_sb[:, tbg : tbg + 1], sc_tiles[tbg], axis=AX
                    )
                mm1 = small.tile([P, 1], F32, tag="mm1")
                nc.vector.reduce_max(mm1, m_sb, axis=AX)
                xs = xps.tile([P, P], F32, tag="xs")
                nc.tensor.transpose(xs[0:1, :], mm1, identf)
                mTs = small.tile([1, P], F32, tag="mTs")
                nc.vector.tensor_copy(mTs, xs[0:1, :])
                M32 = small.tile([1, 32], F32, tag="M32")
                nc.vector.reduce_max(
                    M32, mTs.rearrange("a (j c) -> a c j", j=4), axis=AX
                )
                negM = small.tile([1, 32], F32, tag="negM")
                nc.scalar.mul(out=negM, in_=M32, mul=-1.0)
                negMr = small.tile([1, 4, 32], F32, tag="negMr")
                nc.vector.tensor_copy(
                    negMr, negM.unsqueeze(1).to_broadcast((1, 4, 32))
                )
                xs = xps.tile([P, P], F32, tag="xs")
                nc.tensor.matmul(xs[:, 0:1], negMr, onesf, start=True, stop=True)
                negM128 = small.tile([P, 1], F32, tag="negM128")
                nc.vector.tensor_copy(negM128, xs[:, 0:1])

                ls_sb = small.tile([P, 2], F32, tag="ls")
                for tbg in range(NTBG):
                    nc.scalar.activation(
                        out=p_sb[:, tbg, :],
                        in_=sc_tiles[tbg],
                        func=EXP,
                        bias=negM128,
                        scale=1.0,
                        accum_out=ls_sb[:, tbg : tbg + 1],
                    )
                ls1 = small.tile([P, 1], F32, tag="ls1")
                nc.vector.reduce_sum(ls1, ls_sb, axis=AX)
                xs = xps.tile([P, P], F32, tag="xs")
                nc.tensor.transpose(xs[0:1, :], ls1, identf)
                lTs = small.tile([1, P], F32, tag="lTs")
                nc.vector.tensor_copy(lTs, xs[0:1, :])
                L32 = small.tile([1, 32], F32, tag="L32")
                nc.vector.reduce_sum(
                    L32, lTs.rearrange("a (j c) -> a c j", j=4), axis=AX
                )
                rinv32 = small.tile([1, 32], F32, tag="rinv32")
                nc.vector.reciprocal(rinv32, L32)
                rinvr = small.tile([1, 4, 32], F32, tag="rinvr")
                nc.vector.tensor_copy(
                    rinvr, rinv32.unsqueeze(1).to_broadcast((1, 4, 32))
                )
                xs = xps.tile([P, P], F32, tag="xs")
                nc.tensor.matmul(xs[:, 0:1], rinvr, onesf, start=True, stop=True)
                rinv128 = small.tile([P, 1], F32, tag="rinv128")
                nc.vector.tensor_copy(rinv128, xs[:, 0:1])
                return p_sb, rinv128

            def emit_pt(b, p_sb):
                """p natural (t on partitions) via PE transposes."""
                ptT = []
                for tbg in range(NTBG):
                    pt_ps = xps.tile([P, 4, P], F16, tag="ptT")
                    for i in range(4):
                        nc.tensor.transpose(
                            pt_ps[:, i, :],
                            p_sb[:, tbg, i * P : (i + 1) * P],
                            ident16,
                        )
                    pt_sb = stage.tile([P, 4, P], F16, tag=f"ptT{tbg}")
                    nc.vector.tensor_copy(pt_sb, pt_ps)
                    ptT.append(pt_sb)
                return ptT

            def emit_r(b, ptT):
                """r[h, z] col-tiled over z-quarters; rt = r^T chunks."""
                r_ps = rp.tile([P, 256], F32, tag="r")
                nmm = 0
                for tbg in range(NTBG):
                    for i in range(4):
                        for jt in range(4):
                            gi = (tbg * 4 + jt) * 4 + i
                            for j in range(4):
                                nc.tensor.matmul(
                                    r_ps[32 * j : 32 * j + 16, :],
                                    ptT[tbg][:, i, 32 * jt : 32 * jt + 16],
                                    a_sbs[b][:, gi, j * 256 : (j + 1) * 256],
                                    start=(nmm == 0),
                                    stop=(nmm == 31),
                                    tile_position=(0, 32 * j),
                                )
                            nmm += 1
                r16 = stage.tile([P, 256], F16, tag="r16")
                nc.vector.tensor_copy(r16, r_ps)
                rT_ps = xps.tile([P, 4, P], F16, tag="ptT")
                for half in range(2):
                    nc.tensor.transpose(
                        rT_ps[:, half, :],
                        r16[:, half * P : (half + 1) * P],
                        ident16,
                    )
                rt_sb = stage.tile([P, 2, P], F16, tag="rt")
                nc.vector.tensor_copy(rt_sb, rT_ps[:, 0:2, :])
                return rt_sb

            def emit_ctx(b, rt_sb, rinv128):
                cf_ps = cfp.tile([P, 256], F32, tag="cf")
                for zc in range(ZC):
                    half, zq = zc % 2, zc // 2
                    for j in range(4):
                        nc.tensor.matmul(
                            cf_ps[32 * j : 32 * j + 16, :],
                            rt_sb[:, half, 32 * zq : 32 * zq + 16],
                            wv_sb[:, zc, j * 256 : (j + 1) * 256],
                            start=(zc == 0),
                            stop=(zc == ZC - 1),
                            tile_position=(0, 32 * j),
                        )
                ctxm = stage.tile([P, 256], F32, tag="ctxm")
                nc.vector.tensor_tensor(ctxm, cf_ps, dmask_sb, MULT)
                ctxr = stage.tile([P, DK], F32, tag="ctxr")
                nc.vector.reduce_sum(
                    ctxr, ctxm.rearrange("p (g d) -> p d g", d=DK), axis=AX
                )
                ctxs = stage.tile([P, DK], F32, tag="ctxs")
                nc.vector.tensor_scalar_mul(out=ctxs, in0=ctxr, scalar1=rinv128)
                nc.vector.tensor_add(out=ctxs, in0=ctxs, in1=bv_sb)
                outv = out[b].rearrange("(h d) -> h d", h=H)
                for j in range(4):
                    nc.sync.dma_start(
                        out=outv[4 * j : 4 * j + 4, :],
                        in_=ctxs[36 * j : 36 * j + 4, :],
                    )

            # ------------- interleaved schedule ----------------------------
            sc0 = [emit_scores(0, tbg, emit_fills(0, tbg)) for tbg in range(NTBG)]
            p0, rinv0 = emit_softmax(0, sc0)
            at10 = emit_fills(1, 0)
            sc1_0 = emit_scores(1, 0, at10)
            ptT0 = emit_pt(0, p0)
            rt0 = emit_r(0, ptT0)
            emit_ctx(0, rt0, rinv0)
            at11 = emit_fills(1, 1)
            sc1_1 = emit_scores(1, 1, at11)
            p1, rinv1 = emit_softmax(1, [sc1_0, sc1_1])
            ptT1 = emit_pt(1, p1)
            rt1 = emit_r(1, ptT1)
            emit_ctx(1, rt1, rinv1)

    nc.finalize()
    return nc


_NC_CACHE = {}


def _get_nc():
    if "nc" not in _NC_CACHE:
        _NC_CACHE["nc"] = build_nc()
    return _NC_CACHE["nc"]


def prep_inputs(o_all, o_last, Wk, Wv, Wq, bk, bv, bq):
    """Host-side shard + layout prep. Returns per-core input maps."""
    o_all = np.asarray(o_all, dtype=np.float32)
    o_last = np.asarray(o_last, dtype=np.float32)
    Wk = np.asarray(Wk, dtype=np.float32)
    Wv = np.asarray(Wv, dtype=np.float32)
    Wq = np.asarray(Wq, dtype=np.float32)
    bv = np.asarray(bv, dtype=np.float32)
    bq = np.asarray(bq, dtype=np.float32)

    # q for all batches, then wkq[z, h] = sum_d Wk[h,z,d] q[h,d]
    wq_flat = Wq.transpose(1, 0, 2).reshape(Z, Z)
    q_all = o_last[:, 0, :] @ wq_flat + bq.reshape(Z)          # [B, Z]
    wkq_all = np.einsum(
        "hzd,bhd->bzh", Wk, q_all.reshape(B, H, DK), optimize=True
    )                                                           # [B, Z, H]

    wv_flat = Wv.transpose(1, 0, 2).reshape(Z, Z)
    wv16 = np.ascontiguousarray(
        wv_flat.reshape(ZC, P, Z).transpose(1, 0, 2)
    ).astype(np.float16)

    bv128 = np.zeros((P, DK), dtype=np.float32)
    dmask = np.zeros((P, 256), dtype=np.float32)
    for h in range(H):
        j, r = h // 4, h % 4
        bv128[36 * j + r] = bv[h]
        dmask[32 * j + h, DK * r : DK * (r + 1)] = 1.0

    in_maps = []
    for c in range(NCORES):
        sl = slice(c * BLOC, (c + 1) * BLOC)
        wkq16 = np.ascontiguousarray(
            wkq_all[sl].reshape(BLOC, ZC, P, H).transpose(0, 2, 1, 3)
        ).astype(np.float16)
        in_maps.append(
            {
                "o16": o_all[sl].astype(np.float16),
                "Wv16": wv16,
                "wkq16": wkq16,
                "bv128": bv128,
                "dmask": dmask,
            }
        )
    return in_maps


def kernel(o_all, o_last, Wk, Wv, Wq, bk, bv, bq, _trace=False, _trace_kwargs=None):
    nc = _get_nc()
    in_maps = prep_inputs(o_all, o_last, Wk, Wv, Wq, bk, bv, bq)
    res = run_bass_kernel_spmd(
        nc, in_maps, core_ids=list(range(NCORES)), trace=_trace,
        **(_trace_kwargs or {}),
    )
    outs = [r["out"] for r in res.results]
    full = np.concatenate(outs, axis=0).reshape(B, 1, Z)
    if _trace:
        kernel.last_result = res
    return full


# revision 16
# speedup vs baseline: 2.3141x; 1.0557x over previous
"""MultiHeadTimeDimensionAttention kernel for Trainium2 (8 NeuronCores).

Math (per batch b):
  q[h,d]      = o_last[b] . Wq[h,:,d] + bq[h,d]          (host, 0.4% of FLOPs)
  wkq[z,h]    = sum_d Wk[h,z,d] q[h,d]                   (host)
  scores[t,h] = sum_z o_all[b,t,z] * wkq[z,h]            (device, bk drops: softmax-invariant)
  p = exp(scores - max_t), L = sum_t p
  r[h,z]      = sum_t p[t,h] * o_all[b,t,z]
  ctx[h,d]    = (sum_z r[h,z] Wv[h,z,d]) / L[h] + bv[h,d]

Device layout: scores^T kept in a (tb,h)-packed [128, 512] PSUM layout via
column-tiled (tile_position) M=16 matmuls, 4 t-blocks concurrent on the PE
array.  Softmax runs at full 128-partition parallelism; cross-partition
head reductions go through tiny PE transposes + a K=1 broadcast matmul.
A^T tiles are produced on-PE from the (single) natural-layout copy of
o_all; fp16 everywhere on the PE, fp32 PSUM/softmax.

The two batches per core are software-pipelined: batch 1's transpose fills
and score matmuls are emitted between batch 0's stages so the PE stays busy
during batch 0's softmax and the DMA stream stays ahead of compute.

Sharding: data-parallel over B; each core handles B/8 = 2 batches.
"""

import numpy as np

import concourse.bacc as bacc
import concourse.tile as tile
import concourse.mybir as mybir
from concourse.bass_utils import run_bass_kernel_spmd
from concourse.masks import make_identity

B, T, Z, H, DK = 16, 4096, 1024, 16, 64
P = 128
NCORES = 8
BLOC = B // NCORES          # batches per core
ZC = Z // P                 # 8 z-chunks
TB = 512                    # t-block (one PSUM bank column span)
NTBG = 2                    # two groups of 4 t-blocks per batch
F32 = mybir.dt.float32
F16 = mybir.dt.float16
EXP = mybir.ActivationFunctionType.Exp
AX = mybir.AxisListType.X
MULT = mybir.AluOpType.mult


def build_nc():
    nc = bacc.Bacc(None, target_bir_lowering=False)

    o16 = nc.declare_dram_parameter("o16", [BLOC, T, Z], F16, isOutput=False)
    wv16 = nc.declare_dram_parameter("Wv16", [P, ZC, Z], F16, isOutput=False)
    wkq16 = nc.declare_dram_parameter("wkq16", [BLOC, P, ZC, H], F16, isOutput=False)
    bv128 = nc.declare_dram_parameter("bv128", [P, DK], F32, isOutput=False)
    dmask = nc.declare_dram_parameter("dmask", [P, 256], F32, isOutput=False)
    out = nc.declare_dram_parameter("out", [BLOC, Z], F32, isOutput=True)

    with tile.TileContext(nc) as tc:
        with (
            tc.tile_pool(name="const", bufs=1) as const,
            tc.tile_pool(name="abuf", bufs=2) as abuf,
            tc.tile_pool(name="atbuf", bufs=1) as atbuf,
            tc.tile_pool(name="stage", bufs=2) as stage,
            tc.tile_pool(name="small", bufs=2) as small,
            tc.tile_pool(name="scp", bufs=2, space="PSUM") as scp,
            tc.tile_pool(name="atp", bufs=2, space="PSUM") as atp,
            tc.tile_pool(name="xps", bufs=1, space="PSUM") as xps,
            tc.tile_pool(name="rp", bufs=1, space="PSUM") as rp,
            tc.tile_pool(name="cfp", bufs=1, space="PSUM") as cfp,
        ):
            ident16 = const.tile([P, P], F16)
            make_identity(nc, ident16)
            identf = const.tile([P, P], F32)
            make_identity(nc, identf)
            onesf = const.tile([1, 1], F32)
            nc.vector.memset(onesf, 1.0)
            negones = const.tile([1, 1], F32)
            nc.vector.memset(negones, -1.0)

            # ------------- DMA schedule (single sync ring, FIFO starts) -----
            # tiny first, then b0 stream, then b1 stream with wv interleaved
            wkq_sb = []
            for b in range(BLOC):
                wkq_b = const.tile([P, ZC, H], F16, tag=f"wkq{b}")
                nc.sync.dma_start(out=wkq_b, in_=wkq16[b])
                wkq_sb.append(wkq_b)
            dmask_sb = const.tile([P, 256], F32)
            nc.sync.dma_start(out=dmask_sb, in_=dmask[:])
            bv_sb = const.tile([P, DK], F32)
            nc.sync.dma_start(out=bv_sb, in_=bv128[:])

            wv_sb = const.tile([P, ZC, Z], F16)
            a_sbs = []

            def load_blocks(b, blks):
                for blk in blks:
                    nc.sync.dma_start(
                        out=a_sbs[b][:, blk * 4 : (blk + 1) * 4, :],
                        in_=o16[b, blk * 512 : (blk + 1) * 512, :].rearrange(
                            "(i zp) z -> zp i z", zp=P
                        ),
                    )

            for b in range(BLOC):
                a_sb = abuf.tile([P, 32, Z], F16, tag="a", name=f"a_sb{b}")
                a_sbs.append(a_sb)
            load_blocks(0, range(8))
            load_blocks(1, range(8))
            for zc in range(ZC):
                nc.sync.dma_start(out=wv_sb[:, zc, :], in_=wv16[:, zc, :])

            # ------------- stage emitters ----------------------------------
            def emit_fills(b, tbg, js=range(4), at_sb=None):
                """A^T tiles for t-blocks tbg*4..tbg*4+3 via PE transposes."""
                if at_sb is None:
                    at_sb = atbuf.tile([P, 4, ZC, TB], F16, tag="at")
                nfill = 0
                for j in js:
                    for zcp in range(4):
                        at_ps = atp.tile([P, 8, P], F16, tag="atp")
                        for zz in range(2):
                            zc = 2 * zcp + zz
                            for i in range(4):
                                gi = (tbg * 4 + j) * 4 + i
                                nc.tensor.transpose(
                                    at_ps[:, 4 * zz + i, :],
                                    a_sbs[b][:, gi, zc * P : (zc + 1) * P],
                                    ident16,
                                )
                        eng = nc.scalar if nfill % 3 == 2 else nc.vector
                        cp = eng.copy if nfill % 3 == 2 else eng.tensor_copy
                        cp(
                            out=at_sb[:, j, 2 * zcp : 2 * zcp + 2, :],
                            in_=at_ps.rearrange("p (zz i) c -> p zz (i c)", zz=2),
                        )
                        nfill += 1
                return at_sb

            def emit_scores(b, tbg, at_sb):
                sc_ps = scp.tile([P, TB], F32, tag="sc")
                for zc in range(ZC):
                    for j in range(4):
                        nc.tensor.matmul(
                            sc_ps[32 * j : 32 * j + 16, :],
                            wkq_sb[b][:, zc, :],
                            at_sb[:, j, zc, :],
                            start=(zc == 0),
                            stop=(zc == ZC - 1),
                            tile_position=(0, 32 * j),
                        )
                return sc_ps

            def emit_sm_maxes(b, sc_tiles):
                """per-partition chunk maxes (vector engine only)."""
                m_sb = small.tile([P, 2], F32, tag="m")
                for tbg in range(NTBG):
                    nc.vector.reduce_max(
                        m_sb[:, tbg : tbg + 1], sc_tiles[tbg], axis=AX
                    )
                mm1 = small.tile([P, 1], F32, tag="mm1")
                nc.vector.reduce_max(mm1, m_sb, axis=AX)
                return mm1

            def emit_sm_exp(b, sc_tiles, mm1):
                """per-head max combine + exp; scores are [32*(tb%4)+h, 512]."""
                p_sb = stage.tile([P, NTBG, TB], F16, tag="p")
                xs = xps.tile([P, P], F32, tag="xs")
                nc.tensor.transpose(xs[0:1, :], mm1, identf)
                M32 = small.tile([1, 32], F32, tag="M32")
                nc.vector.reduce_max(
                    M32, xs[0:1, :].rearrange("a (j c) -> a c j", j=4), axis=AX
                )
                Mr = small.tile([1, 4, 32], F32, tag="Mr")
                nc.vector.tensor_copy(Mr, M32.unsqueeze(1).to_broadcast((1, 4, 32)))
                xs = xps.tile([P, P], F32, tag="xs")
                nc.tensor.matmul(xs[:, 0:1], Mr, negones, start=True, stop=True)
                negM128 = small.tile([P, 1], F32, tag="negM128")
                nc.vector.tensor_copy(negM128, xs[:, 0:1])
                ls_sb = small.tile([P, 2], F32, tag="ls")
                for tbg in range(NTBG):
                    nc.scalar.activation(
                        out=p_sb[:, tbg, :],
                        in_=sc_tiles[tbg],
                        func=EXP,
                        bias=negM128,
                        scale=1.0,
                        accum_out=ls_sb[:, tbg : tbg + 1],
                    )
                return p_sb, ls_sb

            def emit_sm_l(b, ls_sb):
                """L = per-head sum of chunk sums; rinv128 = 1/L per partition."""
                ls1 = small.tile([P, 1], F32, tag="ls1")
                nc.vector.reduce_sum(ls1, ls_sb, axis=AX)
                xs = xps.tile([P, P], F32, tag="xs")
                nc.tensor.transpose(xs[0:1, :], ls1, identf)
                L32 = small.tile([1, 32], F32, tag="L32")
                nc.vector.reduce_sum(
                    L32, xs[0:1, :].rearrange("a (j c) -> a c j", j=4), axis=AX
                )
                rinv32 = small.tile([1, 32], F32, tag="rinv32")
                nc.vector.reciprocal(rinv32, L32)
                rinvr = small.tile([1, 4, 32], F32, tag="rinvr")
                nc.vector.tensor_copy(
                    rinvr, rinv32.unsqueeze(1).to_broadcast((1, 4, 32))
                )
                xs = xps.tile([P, P], F32, tag="xs")
                nc.tensor.matmul(xs[:, 0:1], rinvr, onesf, start=True, stop=True)
                rinv128 = small.tile([P, 1], F32, tag="rinv128")
                nc.vector.tensor_copy(rinv128, xs[:, 0:1])
                return rinv128

            def emit_pt(b, p_sb):
                """p natural (t on partitions) via PE transposes."""
                ptT = []
                for tbg in range(NTBG):
                    pt_ps = xps.tile([P, 4, P], F16, tag="ptT")
                    for i in range(4):
                        nc.tensor.transpose(
                            pt_ps[:, i, :],
                            p_sb[:, tbg, i * P : (i + 1) * P],
                            ident16,
                        )
                    pt_sb = stage.tile([P, 4, P], F16, tag=f"ptT{tbg}")
                    nc.vector.tensor_copy(pt_sb, pt_ps)
                    ptT.append(pt_sb)
                return ptT

            def emit_r(b, ptT):
                """r[h, z] col-tiled over z-quarters; rt = r^T chunks."""
                r_ps = rp.tile([P, 256], F32, tag="r")
                nmm = 0
                for tbg in range(NTBG):
                    for i in range(4):
                        for jt in range(4):
                            gi = (tbg * 4 + jt) * 4 + i
                            for j in range(4):
                                nc.tensor.matmul(
                                    r_ps[32 * j : 32 * j + 16, :],
                                    ptT[tbg][:, i, 32 * jt : 32 * jt + 16],
                                    a_sbs[b][:, gi, j * 256 : (j + 1) * 256],
                                    start=(nmm == 0),
                                    stop=(nmm == 31),
                                    tile_position=(0, 32 * j),
                                )
                            nmm += 1
                r16 = stage.tile([P, 256], F16, tag="r16")
                nc.vector.tensor_copy(r16, r_ps)
                rT_ps = xps.tile([P, 4, P], F16, tag="ptT")
                for half in range(2):
                    nc.tensor.transpose(
                        rT_ps[:, half, :],
                        r16[:, half * P : (half + 1) * P],
                        ident16,
                    )
                rt_sb = stage.tile([P, 2, P], F16, tag="rt")
                nc.vector.tensor_copy(rt_sb, rT_ps[:, 0:2, :])
                return rt_sb

            def emit_ctx(b, rt_sb, rinv128):
                cf_ps = cfp.tile([P, 256], F32, tag="cf")
                for zc in range(ZC):
                    half, zq = zc % 2, zc // 2
                    for j in range(4):
                        nc.tensor.matmul(
                            cf_ps[32 * j : 32 * j + 16, :],
                            rt_sb[:, half, 32 * zq : 32 * zq + 16],
                            wv_sb[:, zc, j * 256 : (j + 1) * 256],
                            start=(zc == 0),
                            stop=(zc == ZC - 1),
                            tile_position=(0, 32 * j),
                        )
                ctxm = stage.tile([P, 256], F32, tag="ctxm")
                nc.vector.tensor_tensor(ctxm, cf_ps, dmask_sb, MULT)
                ctxr = stage.tile([P, DK], F32, tag="ctxr")
                nc.vector.reduce_sum(
                    ctxr, ctxm.rearrange("p (g d) -> p d g", d=DK), axis=AX
                )
                ctxs = stage.tile([P, DK], F32, tag="ctxs")
                nc.vector.tensor_scalar_mul(out=ctxs, in0=ctxr, scalar1=rinv128)
                nc.vector.tensor_add(out=ctxs, in0=ctxs, in1=bv_sb)
                outv = out[b].rearrange("(h d) -> h d", h=H)
                for j in range(4):
                    nc.sync.dma_start(
                        out=outv[4 * j : 4 * j + 4, :],
                        in_=ctxs[36 * j : 36 * j + 4, :],
                    )

            # ------------- interleaved schedule ----------------------------
            # PE FIFO: F00 S00 F01 S01 [sm0] F10j0 [sm0-pe] F10j1-3 S10
            #          F11 S11 PT0 R0 CT0 [sm1 overlaps on V/S] PT1 R1 CT1
            sc0 = [emit_scores(0, tbg, emit_fills(0, tbg)) for tbg in range(NTBG)]
            mm1_0 = emit_sm_maxes(0, sc0)
            at10 = emit_fills(1, 0, js=range(0, 1))
            p0, ls0 = emit_sm_exp(0, sc0, mm1_0)
            rinv0 = emit_sm_l(0, ls0)
            emit_fills(1, 0, js=range(1, 4), at_sb=at10)
            sc1_0 = emit_scores(1, 0, at10)
            at11 = emit_fills(1, 1)
            sc1_1 = emit_scores(1, 1, at11)
            mm1_1 = emit_sm_maxes(1, [sc1_0, sc1_1])
            ptT0 = emit_pt(0, p0)
            rt0 = emit_r(0, ptT0)
            p1, ls1 = emit_sm_exp(1, [sc1_0, sc1_1], mm1_1)
            rinv1 = emit_sm_l(1, ls1)
            emit_ctx(0, rt0, rinv0)
            ptT1 = emit_pt(1, p1)
            rt1 = emit_r(1, ptT1)
            emit_ctx(1, rt1, rinv1)

    nc.finalize()
    return nc


_NC_CACHE = {}


def _get_nc():
    if "nc" not in _NC_CACHE:
        _NC_CACHE["nc"] = build_nc()
    return _NC_CACHE["nc"]


def prep_inputs(o_all, o_last, Wk, Wv, Wq, bk, bv, bq):
    """Host-side shard + layout prep. Returns per-core input maps."""
    o_all = np.asarray(o_all, dtype=np.float32)
    o_last = np.asarray(o_last, dtype=np.float32)
    Wk = np.asarray(Wk, dtype=np.float32)
    Wv = np.asarray(Wv, dtype=np.float32)
    Wq = np.asarray(Wq, dtype=np.float32)
    bv = np.asarray(bv, dtype=np.float32)
    bq = np.asarray(bq, dtype=np.float32)

    # q for all batches, then wkq[z, h] = sum_d Wk[h,z,d] q[h,d]
    wq_flat = Wq.transpose(1, 0, 2).reshape(Z, Z)
    q_all = o_last[:, 0, :] @ wq_flat + bq.reshape(Z)          # [B, Z]
    wkq_all = np.einsum(
        "hzd,bhd->bzh", Wk, q_all.reshape(B, H, DK), optimize=True
    )                                                           # [B, Z, H]

    wv_flat = Wv.transpose(1, 0, 2).reshape(Z, Z)
    wv16 = np.ascontiguousarray(
        wv_flat.reshape(ZC, P, Z).transpose(1, 0, 2)
    ).astype(np.float16)

    bv128 = np.zeros((P, DK), dtype=np.float32)
    dmask = np.zeros((P, 256), dtype=np.float32)
    for h in range(H):
        j, r = h // 4, h % 4
        bv128[36 * j + r] = bv[h]
        dmask[32 * j + h, DK * r : DK * (r + 1)] = 1.0

    in_maps = []
    for c in range(NCORES):
        sl = slice(c * BLOC, (c + 1) * BLOC)
        wkq16 = np.ascontiguousarray(
            wkq_all[sl].reshape(BLOC, ZC, P, H).transpose(0, 2, 1, 3)
        ).astype(np.float16)
        in_maps.append(
            {
                "o16": o_all[sl].astype(np.float16),
                "Wv16": wv16,
                "wkq16": wkq16,
                "bv128": bv128,
                "dmask": dmask,
            }
        )
    return in_maps


def kernel(o_all, o_last, Wk, Wv, Wq, bk, bv, bq, _trace=False, _trace_kwargs=None):
    nc = _get_nc()
    in_maps = prep_inputs(o_all, o_last, Wk, Wv, Wq, bk, bv, bq)
    res = run_bass_kernel_spmd(
        nc, in_maps, core_ids=list(range(NCORES)), trace=_trace,
        **(_trace_kwargs or {}),
    )
    outs = [r["out"] for r in res.results]
    full = np.concatenate(outs, axis=0).reshape(B, 1, Z)
    if _trace:
        kernel.last_result = res
    return full
